# revision 1
# baseline (speedup 1.0000x reference)
"""Trainium2 Bass kernel for the Antenna message-generation MLP.

Reference computation (per batch b, RF-chain r, antenna u):
    x[b,r,u,:48] = concat(F[b,:,r], sum_u C[b,u,r,:], H[b,u,8r:8r+8], H[b,u,64+8r:64+8r+8])
    out[b,r,u,:] = tanh(relu(relu(x@W1+b1)@W2+b2)@W3+b3)

Strategy: pure data parallelism over batch across 8 NeuronCores (256
batches = 16384 rows per core).  Rows are processed in 512-row subtiles
with activations feature-on-partition; everything runs in fp16 on the PE
(psum fp32), which the 2e-2 rel-err budget easily covers (sim 5e-4).

PE-array tiling (the main win over a flat 128x128 schedule):
  * L1 (contraction 49 incl. folded-bias ones row): 64x128 row tiling --
    two 512-row subtiles stream concurrently on array halves, X^T packed
    at partitions [0:64)+[64:128).  2x fewer PE slots.
  * L2 (512x512): native 128x128, already at the 216ns/512col floor.
  * L3 (16 out features): 128x32 column tiling -- 4 subtiles stream
    concurrently into psum partition bands 32j..32j+16.  4x fewer slots.
  * One tanh ACTIVATE covers all 4 bands; one DVE 32x32 stream transpose
    yields row-major output for 64B-line DMA.

X^T strip layout (partition bases all 32-aligned for engine writes):
    [0:16)=h  [16]=ones  (17:32 garbage, W1 rows zeroed)  [32:48)=c
    [48:64)=F   (second subtile at +64)
C/H land via one merged [128,512] DMA + one DVE 32x32 stream transpose;
c is u-summed by a single tensor_reduce and rejoined with DMA-transposed
F in a 32-row fc tile so one broadcast copy fills c+F per strip.
"""

import sys
import types

import numpy as np

# This image's `antenv` lacks `axon_hooks`; bass_utils imports it when
# BASS_TRACE is set.  Register a no-op stand-in so tracing degrades
# gracefully instead of crashing (real hook installed by test harness).
try:
    import antenv.axon_hooks  # noqa: F401
except ImportError:
    import antenv

    _m = types.ModuleType("antenv.axon_hooks")
    _m._hook = None
    _m.set_axon_ntff_profile_hook = lambda h: setattr(_m, "_hook", h)
    _m.get_axon_ntff_profile_hook = lambda: _m._hook
    sys.modules["antenv.axon_hooks"] = _m
    antenv.axon_hooks = _m

import concourse.bacc as bacc
import concourse.mybir as mybir
import concourse.tile as tile
from concourse.bass_utils import run_bass_kernel_spmd

F32 = mybir.dt.float32
F16 = mybir.dt.float16

N_CORES = 8
B_FULL = 2048
B_SH = B_FULL // N_CORES    # 256 batches per core
U = 8
R = 8
M = 16
FDIM = 16
H1 = 512
H2 = 512

BG = 16                     # batches per build chunk (1024 rows)
NCH = B_SH // BG            # 16 chunks per core
TILE = 512                  # rows per subtile / psum bank of fp32

_CACHE = {}


def _build():
    nc = bacc.Bacc("TRN2", target_bir_lowering=False, debug=False)

    C_ext = nc.dram_tensor("C", [B_SH, U, R, M], F32, kind="ExternalInput")
    F_ext = nc.dram_tensor("F", [B_SH, FDIM, R], F32, kind="ExternalInput")
    H_ext = nc.dram_tensor("H", [B_SH, U, 2 * 64], F32, kind="ExternalInput")
    W1_ext = nc.dram_tensor("W1", [48, H1], F32, kind="ExternalInput")
    b1_ext = nc.dram_tensor("b1", [H1], F32, kind="ExternalInput")
    W2_ext = nc.dram_tensor("W2", [H1, H2], F32, kind="ExternalInput")
    b2_ext = nc.dram_tensor("b2", [H2], F32, kind="ExternalInput")
    W3_ext = nc.dram_tensor("W3", [H2, M], F32, kind="ExternalInput")
    b3_ext = nc.dram_tensor("b3", [M], F32, kind="ExternalInput")
    # row 0 = 1.0 (folded-bias ones row), rows 1:16 = 0 (pad rows -- must
    # be finite zeros: L1 contracts them against zero weights and
    # 0*inf-garbage would poison the psum with NaNs)
    init_ext = nc.dram_tensor("initrows", [16, TILE], F16, kind="ExternalInput")
    out_ext = nc.dram_tensor("out", [B_SH, R, U, M], F32, kind="ExternalOutput")

    out_rows = out_ext.ap().rearrange("b r u m -> (b r u) m")  # [16384, 16]

    relu = mybir.ActivationFunctionType.Relu
    tanh = mybir.ActivationFunctionType.Tanh
    axis_x = mybir.AxisListType.X
    op_add = mybir.AluOpType.add

    with tile.TileContext(nc) as tc:
        with (
            tc.tile_pool(name="consts", bufs=1) as consts,
            tc.tile_pool(name="loads", bufs=6) as loads,
            tc.tile_pool(name="mts", bufs=2) as mts,
            tc.tile_pool(name="fcs", bufs=6) as fcs,
            tc.tile_pool(name="a1s", bufs=3) as a1p,
            tc.tile_pool(name="a2s", bufs=3) as a2p,
            tc.tile_pool(name="outs", bufs=2) as outs,
            tc.tile_pool(name="p1", bufs=4, space="PSUM") as p1p,
            tc.tile_pool(name="p2", bufs=2, space="PSUM") as p2p,
            tc.tile_pool(name="py", bufs=1, space="PSUM") as pyp,
        ):
            # ---- PE warm-up: keep HAM busy through the input ramp ------
            wtile = consts.tile([128, 128], F16)
            nc.gpsimd.memset(wtile[:], 0.0)
            # hoist the ~2.7us ACT table load off the critical path
            wscr = consts.tile([128, 1], F32)
            nc.scalar.activation(wscr[:], wtile[:, 0:1], mybir.ActivationFunctionType.Tanh)
            ps_warm = pyp.tile([128, TILE], F32, tag="pyA")
            for _ in range(224):
                nc.tensor.matmul(
                    ps_warm[:, 0:64], wtile[:], wtile[:, 0:64],
                    start=True, stop=True,
                )

            # ---- chunk-0/1 input DMAs first (ramp critical path) -------
            mpads = []
            fc_tiles = []

            def build_dma(c):
                b0 = c * BG
                mp = loads.tile([128, 512], F32, tag="mpad")
                # c-region: cols 32r + m (m<16)
                nc.sync.dma_start(
                    mp[:, 0:256].rearrange("p (r w) -> p r w", r=R)[:, :, 0:M],
                    C_ext[b0 : b0 + BG].rearrange("b u r m -> (b u) r m"),
                )
                # h-region: cols 256 + 32r + 8i + k
                hp_v = mp[:, 256:512].rearrange("p (r w) -> p r w", r=R)
                h_src = H_ext[b0 : b0 + BG].rearrange(
                    "b u (i r k) -> (b u) i r k", i=2, r=R
                )
                for i in range(2):
                    nc.sync.dma_start(hp_v[:, :, 8 * i : 8 * i + 8], h_src[:, i])
                # F slice straight into fc rows 16:32 (DMA writes any base)
                fc = fcs.tile([32, 128], F32, tag="fc")
                nc.sync.dma_start(
                    fc[16:32, :].rearrange("f (b r) -> f b r", b=BG),
                    F_ext[b0 : b0 + BG].rearrange("b f r -> f b r"),
                )
                mpads.append(mp)
                fc_tiles.append(fc)

            build_dma(0)

            # ---- weights -----------------------------------------------
            # W1' rows: [0:16)=W1.h [16]=b1 [17:32)=0 [32:48)=W1.c
            # [48:64)=W1.F.  Two zero-padded stationaries keep L1 in plain
            # 128x128 mode: w1A = [W1';0] picks the A strip, w1B = [0;W1']
            # the B strip, so the scheduler can of freely interleave L1/L2
            # matmuls without PE-mode switches.
            w1raw = consts.tile([64, H1], F32)
            nc.gpsimd.memset(w1raw[:], 0.0)
            nc.sync.dma_start(w1raw[0:16, :], W1_ext[32:48])
            nc.sync.dma_start(
                w1raw[16:17, :], b1_ext.ap().rearrange("(o n) -> o n", o=1)
            )
            nc.sync.dma_start(w1raw[32:48, :], W1_ext[16:32])
            nc.sync.dma_start(w1raw[48:64, :], W1_ext[0:16])
            w1A = consts.tile([128, H1], F16)
            w1B = consts.tile([128, H1], F16)
            nc.gpsimd.memset(w1A[:], 0.0)
            nc.gpsimd.memset(w1B[:], 0.0)

            def w1_casts():
                nc.vector.tensor_copy(w1A[0:64, :], w1raw[:])
                nc.vector.tensor_copy(w1B[64:128, :], w1raw[:])

            # ones rows (folded L1 bias) -- persistent xt buffers
            xts = []
            for i in range(3):
                xt = consts.tile([128, TILE], F16, tag=f"xt{i}")
                xts.append(xt)
            nc.sync.dma_start(xts[0][16:32, :], init_ext.ap())
            nc.sync.dma_start(xts[0][80:96, :], init_ext.ap())

            build_dma(1)
            for i in (1, 2):
                nc.sync.dma_start(xts[i][16:32, :], init_ext.ap())
                nc.sync.dma_start(xts[i][80:96, :], init_ext.ap())
            build_dma(2)
            build_dma(3)

            # w2/w3 via SWDGE cast DMA (gpsimd queue; overlaps sync queue)
            w2 = consts.tile([128, 4, H2], F16)
            nc.gpsimd.dma_start(
                w2[:], W2_ext.ap().rearrange("(s p) n -> p s n", p=128)
            )
            w3 = consts.tile([128, 4, M], F16)
            nc.gpsimd.dma_start(
                w3[:], W3_ext.ap().rearrange("(s p) m -> p s m", p=128)
            )
            b2 = consts.tile([128, 4], F32)
            nc.sync.dma_start(b2[:], b2_ext.ap().rearrange("(s p) -> p s", p=128))
            b3t = consts.tile([128, 1], F32)
            nc.gpsimd.memset(b3t[:], 0.0)
            for j in range(4):
                nc.sync.dma_start(
                    b3t[32 * j : 32 * j + M, :],
                    b3_ext.ap().rearrange("(m o) -> m o", o=1),
                )

            # ---- per-chunk build / L1 / L2 ------------------------------
            a1_of_chunk = [None] * NCH
            a2_of_chunk = [None] * NCH

            def build_xt(c):
                mp = mpads[c]
                fc = fc_tiles[c]
                xt = xts[c % 3]
                mt = mts.tile([128, 512], F32, tag="mt")
                nc.vector.transpose(mt[:], mp[:])
                # u-sum of c across all four 32-row bands at once
                cr = mts.tile([128, 32], F32, tag="cred")
                nc.vector.tensor_reduce(
                    cr[:],
                    mt[:, 0:256].rearrange("p (rb u) -> p rb u", u=U),
                    axis_x, op_add,
                )
                # c bands -> fc rows 0:16 (cols (b,r) b-major)
                for a in range(4):
                    nc.vector.tensor_copy(
                        fc[0:16, 32 * a : 32 * a + 32].rearrange(
                            "p (b4 r) -> p r b4", b4=4
                        ),
                        cr[32 * a : 32 * a + 16, :].rearrange(
                            "p (r b4) -> p r b4", b4=4
                        ),
                    )
                # h bands -> xt[0:16) / xt[64:80)
                for a in range(4):
                    hb = 0 if a < 2 else 64
                    dst = xt[hb : hb + 16, :].rearrange(
                        "p (b r u) -> p b r u", b=8, r=R
                    )[:, 4 * (a & 1) : 4 * (a & 1) + 4]
                    src = mt[32 * a : 32 * a + 16, 256:512].rearrange(
                        "p (r b4 u) -> p b4 r u", b4=4, u=U
                    )
                    nc.vector.tensor_copy(dst, src)
                # fc ([c;F], 32 rows) broadcast over u -> xt[32:64)/[96:128)
                for half in range(2):
                    nc.vector.tensor_copy(
                        xt[32 + 64 * half : 64 + 64 * half, :].rearrange(
                            "p (b r u) -> p b r u", b=8, r=R
                        ),
                        fc[:, 64 * half : 64 * half + 64]
                        .rearrange("p (b r) -> p b r", b=8)
                        .unsqueeze(3)
                        .broadcast_to((32, 8, R, U)),
                    )

            def l1(c):
                # ---- L1: plain 128x128 with zero-padded stationaries ----
                xt = xts[c % 3]
                a1c = a1p.tile([128, 4, 2, TILE], F16, tag="a1")
                for s in range(4):
                    for half, w1h in ((0, w1A), (1, w1B)):
                        ps1 = p1p.tile([128, TILE], F32, tag="ps1")
                        nc.tensor.matmul(
                            ps1[:],
                            w1h[:, s * 128 : (s + 1) * 128],
                            xt[:],
                            start=True, stop=True,
                        )
                        if s == 1 or (s == 2 and half == 0):
                            nc.scalar.activation(a1c[:, s, half, :], ps1[:], relu)
                        else:
                            nc.vector.tensor_scalar_max(
                                a1c[:, s, half, :], ps1[:], 0.0
                            )
                a1_of_chunk[c] = a1c

            def l2(c):
                # ---- L2: native 128x128 ---------------------------------
                a1c = a1_of_chunk[c]
                a2c = a2p.tile([128, 2, 4, TILE], F16, tag="a2")
                for h in range(2):
                    for t in range(4):
                        ps2 = p2p.tile([128, TILE], F32, tag="ps2")
                        for s in range(4):
                            nc.tensor.matmul(
                                ps2[:],
                                w2[:, s, t * 128 : (t + 1) * 128],
                                a1c[:, s, h, :],
                                start=(s == 0), stop=(s == 3),
                            )
                        nc.scalar.activation(
                            a2c[:, h, t, :], ps2[:], relu, bias=b2[:, t : t + 1]
                        )
                a2_of_chunk[c] = a2c

            yt_of_group = [None] * (NCH // 2)

            def l3_tanh_half(g, half, psy):
                # one 64-partition half: 2 col-tiled subtiles + tanh
                for tt in range(4):
                    for j in (2 * half, 2 * half + 1):
                        a2c = a2_of_chunk[2 * g + j // 2]
                        nc.tensor.matmul(
                            psy[32 * j : 32 * j + M, :],
                            w3[:, tt, :],
                            a2c[:, j % 2, tt, :],
                            start=(tt == 0), stop=(tt == 3),
                            tile_position=(0, 32 * j),
                        )
                pb = 64 * half
                yt = outs.tile([64, TILE], F32, tag=f"yt{half}")
                nc.scalar.activation(
                    yt[:], psy[pb : pb + 64, :], tanh,
                    bias=b3t[pb : pb + 64, 0:1],
                )
                return yt

            def emit_out_half(g, half, yt):
                ytT = outs.tile([64, TILE], F32, tag=f"ytT{half}")
                nc.vector.transpose(ytT[:], yt[:])
                for jj in range(2):
                    row0 = (4 * g + 2 * half + jj) * TILE
                    eng = nc.sync if jj == 0 else nc.scalar
                    eng.dma_start(
                        out_rows[row0 : row0 + TILE].rearrange(
                            "(k c) m -> c k m", c=32
                        ),
                        ytT[32 * jj : 32 * jj + 32, :].rearrange(
                            "p (k i) -> p k i", k=16
                        )[:, :, 0:M],
                    )

            psy_of_group = [None] * (NCH // 2)

            def l3_mm(g):
                # ---- L3: 128x32 col-tiled over 4 subtiles ---------------
                # j 0/1 -> psyA bands 0,32 ; j 2/3 -> psyB bands 64,96
                psyA = pyp.tile([128, TILE], F32, tag="pyA")
                psyB = pyp.tile([128, TILE], F32, tag="pyB")
                for tt in range(4):
                    for j in range(4):
                        a2c = a2_of_chunk[2 * g + j // 2]
                        psy = psyA if j < 2 else psyB
                        nc.tensor.matmul(
                            psy[32 * j : 32 * j + M, :],
                            w3[:, tt, :],
                            a2c[:, j % 2, tt, :],
                            start=(tt == 0), stop=(tt == 3),
                            tile_position=(0, 32 * j),
                        )
                psy_of_group[g] = (psyA, psyB)

            def tanh_g(g):
                # deferred into the next group body: keeps the scalar FIFO
                # from delaying that group's a1 evacs (ps1 WAR -> PE stall)
                yts = []
                for half, psy in ((0, psy_of_group[g][0]), (1, psy_of_group[g][1])):
                    pb = 64 * half
                    yt = outs.tile([64, TILE], F32, tag=f"yt{half}")
                    nc.scalar.activation(
                        yt[:], psy[pb : pb + 64, :], tanh,
                        bias=b3t[pb : pb + 64, 0:1],
                    )
                    yts.append(yt)
                yt_of_group[g] = yts

            def emit_out(g):
                # deferred a group so next builds precede it in DVE's FIFO
                for half in range(2):
                    ytT = outs.tile([64, TILE], F32, tag=f"ytT{half}")
                    nc.vector.transpose(ytT[:], yt_of_group[g][half][:])
                    for jj in range(2):
                        row0 = (4 * g + 2 * half + jj) * TILE
                        nc.sync.dma_start(
                            out_rows[row0 : row0 + TILE].rearrange(
                                "(k c) m -> c k m", c=32
                            ),
                            ytT[32 * jj : 32 * jj + 32, :].rearrange(
                                "p (k i) -> p k i", k=16
                            )[:, :, 0:M],
                        )

            for g in range(NCH // 2):
                c0, c1 = 2 * g, 2 * g + 1
                build_xt(c0)
                if g == 0:
                    w1_casts()
                l1(c0)
                if g == 0:
                    # fill the first L1->L2 evac-latency bubble (no prior
                    # group's L2 work exists yet to hide it)
                    for _ in range(32):
                        nc.tensor.matmul(
                            ps_warm[:, 0:64], wtile[:], wtile[:, 0:64],
                            start=True, stop=True,
                        )
                build_xt(c1)
                l1(c1)
                if g > 0:
                    tanh_g(g - 1)
                    emit_out(g - 1)
                if c0 + 4 < NCH:
                    build_dma(c0 + 4)
                l2(c0)
                if c1 + 4 < NCH:
                    build_dma(c1 + 4)
                l2(c1)
                if g < NCH // 2 - 1:
                    l3_mm(g)
                else:
                    # final group: finish half A (incl. store) before B's
                    # matmuls end so the kernel tail is one half-chain
                    psyA = pyp.tile([128, TILE], F32, tag="pyA")
                    psyB = pyp.tile([128, TILE], F32, tag="pyB")
                    ytA = l3_tanh_half(g, 0, psyA)
                    ytB = l3_tanh_half(g, 1, psyB)
                    emit_out_half(g, 0, ytA)
                    emit_out_half(g, 1, ytB)

    nc.compile()
    return nc


def _get_nc():
    if "nc" not in _CACHE:
        _CACHE["nc"] = _build()
    return _CACHE["nc"]


def run(inputs, trace=False):
    nc = _get_nc()
    np_in = {k: np.ascontiguousarray(np.asarray(v, dtype=np.float32))
             for k, v in inputs.items()}
    initrows = np.zeros((16, TILE), dtype=np.float16)
    initrows[0, :] = 1.0
    in_maps = []
    for i in range(N_CORES):
        sl = slice(i * B_SH, (i + 1) * B_SH)
        in_maps.append({
            "C": np_in["C"][sl],
            "F": np_in["F"][sl],
            "H": np_in["H"][sl],
            "W1": np_in["W1"], "b1": np_in["b1"],
            "W2": np_in["W2"], "b2": np_in["b2"],
            "W3": np_in["W3"], "b3": np_in["b3"],
            "initrows": initrows,
        })
    res = run_bass_kernel_spmd(nc, in_maps, list(range(N_CORES)), trace=trace)
    out = np.concatenate([res.results[i]["out"] for i in range(N_CORES)], axis=0)
    return out, res


def kernel(**inputs):
    out, _ = run(inputs, trace=False)
    return out



# revision 8
# speedup vs baseline: 1.0696x; 1.0696x over previous
"""Trainium2 Bass kernel for the Antenna message-generation MLP.

Reference computation (per batch b, RF-chain r, antenna u):
    x[b,r,u,:48] = concat(F[b,:,r], sum_u C[b,u,r,:], H[b,u,8r:8r+8], H[b,u,64+8r:64+8r+8])
    out[b,r,u,:] = tanh(relu(relu(x@W1+b1)@W2+b2)@W3+b3)

Strategy: pure data parallelism over batch across 8 NeuronCores (256
batches = 16384 rows per core).  Rows are processed in 1024-row chunks
(two 512-row subtiles A/B), activations feature-on-partition, fp16 on
the PE (fp32 psum).

Differences from the previous 198us version:
  * Weights are packed on the HOST into fp16 device layouts (w1p/w2p/
    w3p + one [128,9] bias pack) -- no SWDGE cast DMAs, no on-chip w1
    shuffling, and b1/b2/b3 ride the ACT bias port so the folded-bias
    ones rows disappear (L1 contraction 48 in a 64-row band).
  * L1 is 2-way ROW-TILED: subtile A's X^T at partitions 0:64 with the
    stationary at array rows 0:64, subtile B at 64:128/(64,0).  The two
    64-contraction matmuls run concurrently on disjoint row bands ->
    half the PE slots of the old zero-padded 128x128 scheme.
  * Emission interleaves each L1 pair with two L2 groups of the
    previous chunk so psum-bank WAR never blocks the PE FIFO head.
  * PSUM: L1 2x two-bank pair tiles, L2 3 banks (the old 2-bank L2
    rotation cost +54ns at every 4-MM group boundary), L3 packs its 4
    column bands (partitions 32j) into ONE bank.
  * Evacuations balanced across scalar/ACT and vector/DVE (Pool can't
    read PSUM): scalar 7 L2 evacs + 2 pair evacs + a tanh half per
    chunk, DVE 2 pair evacs + 1 L2 evac + builds/transposes.
  * Ramp: chunk 0-3 input DMAs spread across sync/vector/scalar/gpsimd
    queues; tail: final group's tanh/store of bands 0:64 overlaps the
    last chunk's L2 groups.

X^T strip layout (per 64-partition half):
    [0:16)=h  [16:32)=zeros  [32:48)=c  [48:64)=F
C/H land via one merged [128,512] DMA + one DVE 32x32 stream transpose;
c is u-summed by a single tensor_reduce and rejoined with DMA-transposed
F in a 32-row fc tile so one broadcast copy fills c+F per strip.
"""

import sys
import types

import numpy as np

# This image's `antenv` lacks `axon_hooks`; bass_utils imports it when
# BASS_TRACE is set.  Register a no-op stand-in so tracing degrades
# gracefully instead of crashing (real hook installed by test harness).
try:
    import antenv.axon_hooks  # noqa: F401
except ImportError:
    import antenv

    _m = types.ModuleType("antenv.axon_hooks")
    _m._hook = None
    _m.set_axon_ntff_profile_hook = lambda h: setattr(_m, "_hook", h)
    _m.get_axon_ntff_profile_hook = lambda: _m._hook
    sys.modules["antenv.axon_hooks"] = _m
    antenv.axon_hooks = _m

import concourse.bacc as bacc
import concourse.mybir as mybir
import concourse.tile as tile
from concourse.bass_utils import run_bass_kernel_spmd

F32 = mybir.dt.float32
F16 = mybir.dt.float16

N_CORES = 8
B_FULL = 2048
B_SH = B_FULL // N_CORES    # 256 batches per core
U = 8
R = 8
M = 16
FDIM = 16
H1 = 512
H2 = 512

BG = 16                     # batches per build chunk (1024 rows)
NCH = B_SH // BG            # 16 chunks per core
TILE = 512                  # rows per subtile / psum bank of fp32

N_WARM = 104                # PE warm-up matmuls before first L1 pair
N_FILL = 64                 # pipeline-fill matmuls after chunk 0's pairs

_CACHE = {}


def _build():
    nc = bacc.Bacc("TRN2", target_bir_lowering=False, debug=False)

    C_ext = nc.dram_tensor("C", [B_SH, U, R, M], F32, kind="ExternalInput")
    F_ext = nc.dram_tensor("F", [B_SH, FDIM, R], F32, kind="ExternalInput")
    H_ext = nc.dram_tensor("H", [B_SH, U, 2 * 64], F32, kind="ExternalInput")
    # host-packed weights (see _pack_weights)
    w1_ext = nc.dram_tensor("w1p", [128, H1], F16, kind="ExternalInput")
    w2_ext = nc.dram_tensor("w2p", [128, 4, H2], F16, kind="ExternalInput")
    w3_ext = nc.dram_tensor("w3p", [128, 4, 32], F16, kind="ExternalInput")
    # cols 0:4 = b1 (by s-tile), 4:8 = b2 (by t-tile), 8 = b3 (banded)
    bias_ext = nc.dram_tensor("biasp", [128, 9], F32, kind="ExternalInput")
    out_ext = nc.dram_tensor("out", [B_SH, R, U, M], F32, kind="ExternalOutput")

    out_rows = out_ext.ap().rearrange("b r u m -> (b r u) m")  # [16384, 16]

    relu = mybir.ActivationFunctionType.Relu
    tanh = mybir.ActivationFunctionType.Tanh
    axis_x = mybir.AxisListType.X
    op_add = mybir.AluOpType.add
    op_max = mybir.AluOpType.max

    with tile.TileContext(nc) as tc:
        with (
            tc.tile_pool(name="consts", bufs=1) as consts,
            tc.tile_pool(name="loads", bufs=6) as loads,
            tc.tile_pool(name="mts", bufs=3) as mts,
            tc.tile_pool(name="fcs", bufs=6) as fcs,
            tc.tile_pool(name="a1s", bufs=3) as a1p,
            tc.tile_pool(name="a2s", bufs=4) as a2p,
            tc.tile_pool(name="outs", bufs=2) as outs,
            tc.tile_pool(name="p1", bufs=2, space="PSUM") as p1p,
            tc.tile_pool(name="p2", bufs=3, space="PSUM") as p2p,
            tc.tile_pool(name="py", bufs=1, space="PSUM") as pyp,
        ):
            # ---- persistent tiles --------------------------------------
            w1 = consts.tile([128, H1], F16)
            w2 = consts.tile([128, 4, H2], F16)
            w3 = consts.tile([128, 4, 32], F16)
            biasc = consts.tile([128, 9], F32)
            wtile = consts.tile([128, 128], F16)
            wscr = consts.tile([128, 1], F32)
            xts = [consts.tile([128, TILE], F16, tag=f"xt{i}", name=f"xt{i}")
                   for i in range(4)]

            mpads = []
            fc_tiles = []

            def build_dma(c, qc=None, qh=None, qf=None):
                qc = qc or nc.sync
                qh = qh or nc.sync
                qf = qf or nc.sync
                b0 = c * BG
                mp = loads.tile([128, 512], F32, tag="mpad", name="mpad")
                # the DMAs below only fill the lower 16 cols of each 32-col
                # r-block; zero the upper halves so the full-tile DVE
                # transpose never reads uninitialized SBUF (the transposed
                # garbage bands are discarded, but CoreSim checks reads)
                mpv = mp.rearrange("p (r w m) -> p r w m", r=2 * R, w=2)
                nc.gpsimd.memset(mpv[:, :, 1, :], 0.0)
                # c-region: cols 32r + m (m<16)
                qc.dma_start(
                    mp[:, 0:256].rearrange("p (r w) -> p r w", r=R)[:, :, 0:M],
                    C_ext[b0 : b0 + BG].rearrange("b u r m -> (b u) r m"),
                )
                # h-region: cols 256 + 32r + 8i + k
                hp_v = mp[:, 256:512].rearrange("p (r w) -> p r w", r=R)
                h_src = H_ext[b0 : b0 + BG].rearrange(
                    "b u (i r k) -> (b u) i r k", i=2, r=R
                )
                for i in range(2):
                    qh.dma_start(hp_v[:, :, 8 * i : 8 * i + 8], h_src[:, i])
                # F slice straight into fc rows 16:32 (DMA writes any base)
                fcv = fcs.tile([32, 128], F32, tag="fc", name="fc")
                qf.dma_start(
                    fcv[16:32, :].rearrange("f (b r) -> f b r", b=BG),
                    F_ext[b0 : b0 + BG].rearrange("b f r -> f b r"),
                )
                mpads.append(mp)
                fc_tiles.append(fcv)

            # ---- ramp: all input + weight DMAs first, spread over the
            # three DMA-capable queues (sync/SP, scalar/ACT, gpsimd) ----
            # xt pad rows must be finite zeros once; memsets first on the
            # gpsimd queue so build(0)'s xt writes aren't stuck behind its
            # SWDGE descriptor generation
            nc.gpsimd.memset(wtile[:], 0.0)
            for xt in xts:
                nc.gpsimd.memset(xt[:], 0.0)
            build_dma(0, qc=nc.sync, qh=nc.scalar, qf=nc.scalar)
            nc.sync.dma_start(w1[:], w1_ext.ap())
            build_dma(1, qc=nc.scalar, qh=nc.sync, qf=nc.gpsimd)
            # hoist the ~2.7us ACT table load off the critical path
            nc.scalar.activation(wscr[:], wtile[:, 0:1], tanh)
            nc.scalar.dma_start(w2[:], w2_ext.ap())
            nc.sync.dma_start(w3[:], w3_ext.ap())
            nc.sync.dma_start(biasc[:], bias_ext.ap())
            build_dma(2, qc=nc.sync, qh=nc.sync, qf=nc.gpsimd)
            build_dma(3, qc=nc.gpsimd, qh=nc.scalar, qf=nc.gpsimd)

            # ---- PE warm-up: keep HAM busy through the input ramp ------
            ps_warm = pyp.tile([128, TILE], F32, tag="psy", name="ps_warm")

            def warm(n):
                for _ in range(n):
                    nc.tensor.matmul(
                        ps_warm[:, 0:64], wtile[:], wtile[:, 0:64],
                        start=True, stop=True,
                    )

            warm(N_WARM)

            # ---- per-chunk build ---------------------------------------
            a1_of_chunk = [None] * NCH
            a2_of_chunk = [None] * NCH
            psy_of_group = [None] * (NCH // 2)
            yt_of_group = [None] * (NCH // 2)

            def build_xt(c):
                mp = mpads[c]
                fcv = fc_tiles[c]
                xt = xts[c % 4]
                mt = mts.tile([128, 512], F32, tag="mt", name="mt")
                nc.vector.transpose(mt[:], mp[:])
                # u-sum of c across all four 32-row bands at once
                cr = mts.tile([128, 32], F32, tag="cred", name="cred")
                nc.vector.tensor_reduce(
                    cr[:],
                    mt[:, 0:256].rearrange("p (rb u) -> p rb u", u=U),
                    axis_x, op_add,
                )
                # c bands -> fc rows 0:16 (cols (b,r) b-major)
                for a in range(4):
                    nc.vector.tensor_copy(
                        fcv[0:16, 32 * a : 32 * a + 32].rearrange(
                            "p (b4 r) -> p r b4", b4=4
                        ),
                        cr[32 * a : 32 * a + 16, :].rearrange(
                            "p (r b4) -> p r b4", b4=4
                        ),
                    )
                # h bands -> xt[0:16) / xt[64:80)
                for a in range(4):
                    hb = 0 if a < 2 else 64
                    dst = xt[hb : hb + 16, :].rearrange(
                        "p (b r u) -> p b r u", b=8, r=R
                    )[:, 4 * (a & 1) : 4 * (a & 1) + 4]
                    src = mt[32 * a : 32 * a + 16, 256:512].rearrange(
                        "p (r b4 u) -> p b4 r u", b4=4, u=U
                    )
                    nc.vector.tensor_copy(dst, src)
                # fc ([c;F], 32 rows) broadcast over u -> xt[32:64)/[96:128)
                for half in range(2):
                    nc.vector.tensor_copy(
                        xt[32 + 64 * half : 64 + 64 * half, :].rearrange(
                            "p (b r u) -> p b r u", b=8, r=R
                        ),
                        fcv[:, 64 * half : 64 * half + 64]
                        .rearrange("p (b r) -> p b r", b=8)
                        .unsqueeze(3)
                        .broadcast_to((32, 8, R, U)),
                    )

            # ---- L1: 2-way row-tiled pair ------------------------------
            def pair(c, s):
                xt = xts[c % 4]
                psp = p1p.tile([128, 2, TILE], F32, tag="ps1", name="psp")
                for half in range(2):
                    pb = 64 * half
                    nc.tensor.matmul(
                        psp[:, half, :],
                        w1[pb : pb + 64, s * 128 : (s + 1) * 128],
                        xt[pb : pb + 64, :],
                        start=True, stop=True,
                    )
                return psp

            def evac_pair(c, s, psp, eng="S"):
                a1c = a1_of_chunk[c]
                if eng == "S":
                    nc.scalar.activation(
                        a1c[:, s, :, :], psp[:, :, :], relu,
                        bias=biasc[:, s : s + 1],
                    )
                else:
                    nc.vector.tensor_scalar(
                        a1c[:, s, :, :], psp[:, :, :],
                        biasc[:, s : s + 1], 0.0, op_add, op_max,
                    )

            # ---- L2: group k = (h, t), 4-MM accumulation ---------------
            # (gpsimd/Pool cannot touch PSUM on trn2, so evacuations are
            # spread over scalar/ACT and vector/DVE only)
            def l2_group(c, k, eng="S"):
                h, t = k // 4, k % 4
                a1c = a1_of_chunk[c]
                ps2 = p2p.tile([128, TILE], F32, tag="ps2", name="ps2")
                for s in range(4):
                    nc.tensor.matmul(
                        ps2[:],
                        w2[:, s, t * 128 : (t + 1) * 128],
                        a1c[:, s, h, :],
                        start=(s == 0), stop=(s == 3),
                    )
                a2c = a2_of_chunk[c]
                dst = a2c[:, h, t, :]
                bcol = biasc[:, 4 + t : 5 + t]
                if eng == "S":
                    nc.scalar.activation(dst, ps2[:], relu, bias=bcol)
                else:
                    nc.vector.tensor_scalar(
                        dst, ps2[:], bcol, 0.0, op_add, op_max
                    )

            # ---- L3: 128x32 col-tiled, 4 bands in one psum bank --------
            def l3_part(g, js):
                psy = psy_of_group[g]
                for tt in range(4):
                    for j in js:
                        a2c = a2_of_chunk[2 * g + j // 2]
                        # sim's psum group check is zero-region-coarse; the
                        # four col bands accumulate independently on HW
                        # (per-element has_written), so skip it
                        nc.tensor.matmul(
                            psy[32 * j : 32 * j + 32, :],
                            w3[:, tt, :],
                            a2c[:, j % 2, tt, :],
                            start=(tt == 0), stop=(tt == 3),
                            tile_position=(0, 32 * j),
                            skip_group_check=True,
                        )

            def tanh_half(g, half):
                psy = psy_of_group[g]
                pb = 64 * half
                yt = outs.tile([64, TILE], F32, tag=f"yt{half}", name=f"yt{half}")
                nc.scalar.activation(
                    yt[:], psy[pb : pb + 64, :], tanh,
                    bias=biasc[pb : pb + 64, 8:9],
                )
                if yt_of_group[g] is None:
                    yt_of_group[g] = [None, None]
                yt_of_group[g][half] = yt

            def emit_half(g, half):
                ytT = outs.tile([64, TILE], F32, tag=f"ytT{half}", name=f"ytT{half}")
                nc.vector.transpose(ytT[:], yt_of_group[g][half][:])
                for jj in range(2):
                    row0 = (4 * g + 2 * half + jj) * TILE
                    nc.sync.dma_start(
                        out_rows[row0 : row0 + TILE].rearrange(
                            "(k c) m -> c k m", c=32
                        ),
                        ytT[32 * jj : 32 * jj + 32, :].rearrange(
                            "p (k i) -> p k i", k=16
                        )[:, :, 0:M],
                    )

            # ---- steady-state chunk streams ----------------------------
            # per s-slot: one L1 pair + two L2 groups of the previous
            # chunk.  build_xt(c+1) is emitted mid-stream (s=1) so the DVE
            # FIFO never parks it behind late-psum evacs; tanh halves are
            # split across adjacent chunks (A at s=2 of even chunks, B at
            # s=0 of the following odd chunk, always before that chunk's
            # l3 at s=2 reuses the psy bank).
            L2_ENG = {0: "S", 1: "S", 2: "S", 3: "S",
                      4: "S", 5: "V", 6: "S", 7: "S"}
            build_xt(0)
            for c in range(NCH):
                a1_of_chunk[c] = a1p.tile(
                    [128, 4, 2, TILE], F16, tag="a1", name="a1c"
                )
                a2_of_chunk[c] = a2p.tile(
                    [128, 2, 4, TILE], F16, tag="a2", name="a2c"
                )
                for s in range(4):
                    if s == 0 and c >= 5 and c % 2 == 1:
                        g = (c - 5) // 2
                        tanh_half(g, 1)
                        emit_half(g, 1)
                    psp = pair(c, s)
                    evac_pair(c, s, psp, eng=("V" if s in (0, 2) else "S"))
                    if c >= 1:
                        l2_group(c - 1, 2 * s, L2_ENG[2 * s])
                        l2_group(c - 1, 2 * s + 1, L2_ENG[2 * s + 1])
                    if s == 1 and c + 1 < NCH:
                        build_xt(c + 1)
                    if s == 2:
                        if c >= 4 and c % 2 == 0:
                            g = (c - 4) // 2
                            tanh_half(g, 0)
                            emit_half(g, 0)
                        if c >= 3 and c % 2 == 1:
                            g = (c - 3) // 2
                            psy_of_group[g] = pyp.tile(
                                [128, TILE], F32, tag="psy", name="psy"
                            )
                            l3_part(g, (0, 1, 2, 3))
                    if c == 0 and s == 0:
                        warm(N_FILL)
                if c + 4 < NCH:
                    build_dma(c + 4)

            # ---- drain: l2(15), l3(7) split, tanh(6,7) -----------------
            tanh_half(6, 0)
            tanh_half(6, 1)
            emit_half(6, 0)
            emit_half(6, 1)
            psy_of_group[7] = pyp.tile([128, TILE], F32, tag="psy", name="psyF")
            l2_group(15, 0, "S")
            l2_group(15, 1, "V")
            # bands 0,1 need only chunk 14's a2 -- finish + store half A
            # while the rest of chunk 15's L2 still runs
            l3_part(7, (0, 1))
            tanh_half(7, 0)
            emit_half(7, 0)
            l2_group(15, 2, "S")
            l2_group(15, 3, "V")
            l3_part(7, (2,))
            l2_group(15, 4, "S")
            l2_group(15, 5, "V")
            l2_group(15, 6, "S")
            l2_group(15, 7, "V")
            l3_part(7, (3,))
            tanh_half(7, 1)
            emit_half(7, 1)

    nc.compile()
    return nc


def _pack_weights(np_in):
    W1 = np.asarray(np_in["W1"], np.float32)
    W2 = np.asarray(np_in["W2"], np.float32)
    W3 = np.asarray(np_in["W3"], np.float32)
    b1 = np.asarray(np_in["b1"], np.float32)
    b2 = np.asarray(np_in["b2"], np.float32)
    b3 = np.asarray(np_in["b3"], np.float32)

    # X^T strip rows: [0:16)=h [16:32)=0 [32:48)=c [48:64)=F ; W1 rows are
    # ordered (F 0:16, c 16:32, h 32:48) in the reference
    w1p = np.zeros((128, H1), np.float16)
    w1p[0:16] = W1[32:48]
    w1p[32:48] = W1[16:32]
    w1p[48:64] = W1[0:16]
    w1p[64:128] = w1p[0:64]

    w2p = np.ascontiguousarray(
        W2.reshape(4, 128, H2).transpose(1, 0, 2).astype(np.float16)
    )
    # pad W3 to 32 out cols so each L3 col band writes its full 32 psum
    # partitions (bands 16:32 etc. would otherwise be uninitialized reads
    # for the whole-half tanh; as zeros they tanh to 0 and are dropped)
    w3p = np.zeros((128, 4, 32), np.float16)
    w3p[:, :, 0:M] = W3.reshape(4, 128, M).transpose(1, 0, 2)
    biasp = np.zeros((128, 9), np.float32)
    biasp[:, 0:4] = b1.reshape(4, 128).T
    biasp[:, 4:8] = b2.reshape(4, 128).T
    for j in range(4):
        biasp[32 * j : 32 * j + M, 8] = b3
    return {"w1p": w1p, "w2p": w2p, "w3p": w3p, "biasp": biasp}


def _core_inputs(np_in, i, packed=None):
    if packed is None:
        packed = _pack_weights(np_in)
    sl = slice(i * B_SH, (i + 1) * B_SH)
    return {
        "C": np_in["C"][sl],
        "F": np_in["F"][sl],
        "H": np_in["H"][sl],
        **packed,
    }


def _get_nc():
    if "nc" not in _CACHE:
        _CACHE["nc"] = _build()
    return _CACHE["nc"]


def run(inputs, trace=False):
    nc = _get_nc()
    np_in = {k: np.ascontiguousarray(np.asarray(v, dtype=np.float32))
             for k, v in inputs.items()}
    packed = _pack_weights(np_in)
    in_maps = [_core_inputs(np_in, i, packed) for i in range(N_CORES)]
    res = run_bass_kernel_spmd(nc, in_maps, list(range(N_CORES)), trace=trace)
    out = np.concatenate([res.results[i]["out"] for i in range(N_CORES)], axis=0)
    return out, res


def kernel(**inputs):
    out, _ = run(inputs, trace=False)
    return out


# revision 9
# speedup vs baseline: 1.0978x; 1.0263x over previous
"""Trainium2 Bass kernel for the Antenna message-generation MLP.

Reference computation (per batch b, RF-chain r, antenna u):
    x[b,r,u,:48] = concat(F[b,:,r], sum_u C[b,u,r,:], H[b,u,8r:8r+8], H[b,u,64+8r:64+8r+8])
    out[b,r,u,:] = tanh(relu(relu(x@W1+b1)@W2+b2)@W3+b3)

Strategy: pure data parallelism over batch across 8 NeuronCores (256
batches = 16384 rows per core).  Rows are processed in 1024-row chunks
(two 512-row subtiles A/B), activations feature-on-partition, fp16 on
the PE (fp32 psum).

Differences from the previous 198us version:
  * Weights are packed on the HOST into fp16 device layouts (w1p/w2p/
    w3p + one [128,9] bias pack) -- no SWDGE cast DMAs, no on-chip w1
    shuffling, and b1/b2/b3 ride the ACT bias port so the folded-bias
    ones rows disappear (L1 contraction 48 in a 64-row band).
  * L1 is 2-way ROW-TILED: subtile A's X^T at partitions 0:64 with the
    stationary at array rows 0:64, subtile B at 64:128/(64,0).  The two
    64-contraction matmuls run concurrently on disjoint row bands ->
    half the PE slots of the old zero-padded 128x128 scheme.
  * Emission interleaves each L1 pair with two L2 groups of the
    previous chunk so psum-bank WAR never blocks the PE FIFO head.
  * PSUM: L1 2x two-bank pair tiles, L2 3 banks (the old 2-bank L2
    rotation cost +54ns at every 4-MM group boundary), L3 packs its 4
    column bands (partitions 32j) into ONE bank.
  * Evacuations balanced across scalar/ACT and vector/DVE (Pool can't
    read PSUM): scalar 7 L2 evacs + 2 pair evacs + a tanh half per
    chunk, DVE 2 pair evacs + 1 L2 evac + builds/transposes.
  * Ramp: chunk 0-3 input DMAs spread across sync/vector/scalar/gpsimd
    queues; tail: final group's tanh/store of bands 0:64 overlaps the
    last chunk's L2 groups.

X^T strip layout (per 64-partition half):
    [0:16)=h  [16:32)=zeros  [32:48)=c  [48:64)=F
C/H land via one merged [128,512] DMA + one DVE 32x32 stream transpose;
c is u-summed by a single tensor_reduce and rejoined with DMA-transposed
F in a 32-row fc tile so one broadcast copy fills c+F per strip.
"""

import sys
import types

import numpy as np

# This image's `antenv` lacks `axon_hooks`; bass_utils imports it when
# BASS_TRACE is set.  Register a no-op stand-in so tracing degrades
# gracefully instead of crashing (real hook installed by test harness).
try:
    import antenv.axon_hooks  # noqa: F401
except ImportError:
    import antenv

    _m = types.ModuleType("antenv.axon_hooks")
    _m._hook = None
    _m.set_axon_ntff_profile_hook = lambda h: setattr(_m, "_hook", h)
    _m.get_axon_ntff_profile_hook = lambda: _m._hook
    sys.modules["antenv.axon_hooks"] = _m
    antenv.axon_hooks = _m

import concourse.bacc as bacc
import concourse.mybir as mybir
import concourse.tile as tile
from concourse.bass_utils import run_bass_kernel_spmd

F32 = mybir.dt.float32
F16 = mybir.dt.float16

N_CORES = 8
B_FULL = 2048
B_SH = B_FULL // N_CORES    # 256 batches per core
U = 8
R = 8
M = 16
FDIM = 16
H1 = 512
H2 = 512

BG = 16                     # batches per build chunk (1024 rows)
NCH = B_SH // BG            # 16 chunks per core
TILE = 512                  # rows per subtile / psum bank of fp32

N_WARM = 112                # PE warm-up matmuls before first L1 pair
N_FILL = 96                 # pipeline-fill matmuls after chunk 0's pairs

_CACHE = {}


def _build():
    nc = bacc.Bacc("TRN2", target_bir_lowering=False, debug=False)

    C_ext = nc.dram_tensor("C", [B_SH, U, R, M], F32, kind="ExternalInput")
    F_ext = nc.dram_tensor("F", [B_SH, FDIM, R], F32, kind="ExternalInput")
    H_ext = nc.dram_tensor("H", [B_SH, U, 2 * 64], F32, kind="ExternalInput")
    # host-packed weights (see _pack_weights)
    w1_ext = nc.dram_tensor("w1p", [128, H1], F16, kind="ExternalInput")
    w2_ext = nc.dram_tensor("w2p", [128, 4, H2], F16, kind="ExternalInput")
    w3_ext = nc.dram_tensor("w3p", [128, 4, 32], F16, kind="ExternalInput")
    # cols 0:4 = b1 (by s-tile), 4:8 = b2 (by t-tile), 8 = b3 (banded)
    bias_ext = nc.dram_tensor("biasp", [128, 9], F32, kind="ExternalInput")
    out_ext = nc.dram_tensor("out", [B_SH, R, U, M], F32, kind="ExternalOutput")

    out_rows = out_ext.ap().rearrange("b r u m -> (b r u) m")  # [16384, 16]

    relu = mybir.ActivationFunctionType.Relu
    tanh = mybir.ActivationFunctionType.Tanh
    axis_x = mybir.AxisListType.X
    op_add = mybir.AluOpType.add
    op_max = mybir.AluOpType.max

    with tile.TileContext(nc) as tc:
        with (
            tc.tile_pool(name="consts", bufs=1) as consts,
            tc.tile_pool(name="loads", bufs=6) as loads,
            tc.tile_pool(name="mts", bufs=3) as mts,
            tc.tile_pool(name="fcs", bufs=6) as fcs,
            tc.tile_pool(name="a1s", bufs=3) as a1p,
            tc.tile_pool(name="a2s", bufs=4) as a2p,
            tc.tile_pool(name="outs", bufs=2) as outs,
            tc.tile_pool(name="p1", bufs=2, space="PSUM") as p1p,
            tc.tile_pool(name="p2", bufs=3, space="PSUM") as p2p,
            tc.tile_pool(name="py", bufs=1, space="PSUM") as pyp,
        ):
            # ---- persistent tiles --------------------------------------
            w1 = consts.tile([128, H1], F16)
            w2 = consts.tile([128, 4, H2], F16)
            w3 = consts.tile([128, 4, 32], F16)
            biasc = consts.tile([128, 9], F32)
            wtile = consts.tile([128, 128], F16)
            wscr = consts.tile([128, 1], F32)
            xts = [consts.tile([128, TILE], F16, tag=f"xt{i}", name=f"xt{i}")
                   for i in range(4)]

            mpads = []
            fc_tiles = []

            def build_dma(c, qc=None, qh=None, qf=None):
                qc = qc or nc.sync
                qh = qh or nc.sync
                qf = qf or nc.sync
                b0 = c * BG
                mp = loads.tile([128, 512], F32, tag="mpad", name="mpad")
                # the DMAs below only fill the lower 16 cols of each 32-col
                # r-block; zero the upper halves so the full-tile DVE
                # transpose never reads uninitialized SBUF (the transposed
                # garbage bands are discarded, but CoreSim checks reads)
                mpv = mp.rearrange("p (r w m) -> p r w m", r=2 * R, w=2)
                nc.gpsimd.memset(mpv[:, :, 1, :], 0.0)
                # c-region: cols 32r + m (m<16)
                qc.dma_start(
                    mp[:, 0:256].rearrange("p (r w) -> p r w", r=R)[:, :, 0:M],
                    C_ext[b0 : b0 + BG].rearrange("b u r m -> (b u) r m"),
                )
                # h-region: cols 256 + 32r + 8i + k
                hp_v = mp[:, 256:512].rearrange("p (r w) -> p r w", r=R)
                h_src = H_ext[b0 : b0 + BG].rearrange(
                    "b u (i r k) -> (b u) i r k", i=2, r=R
                )
                for i in range(2):
                    qh.dma_start(hp_v[:, :, 8 * i : 8 * i + 8], h_src[:, i])
                # F slice straight into fc rows 16:32 (DMA writes any base)
                fcv = fcs.tile([32, 128], F32, tag="fc", name="fc")
                qf.dma_start(
                    fcv[16:32, :].rearrange("f (b r) -> f b r", b=BG),
                    F_ext[b0 : b0 + BG].rearrange("b f r -> f b r"),
                )
                mpads.append(mp)
                fc_tiles.append(fcv)

            # ---- ramp: all input + weight DMAs first, spread over the
            # three DMA-capable queues (sync/SP, scalar/ACT, gpsimd) ----
            # xt pad rows must be finite zeros once; memsets first on the
            # gpsimd queue so build(0)'s xt writes aren't stuck behind its
            # SWDGE descriptor generation
            nc.gpsimd.memset(wtile[:], 0.0)
            for xt in xts:
                nc.gpsimd.memset(xt[:], 0.0)
            build_dma(0, qc=nc.sync, qh=nc.scalar, qf=nc.scalar)
            nc.sync.dma_start(w1[:], w1_ext.ap())
            build_dma(1, qc=nc.scalar, qh=nc.sync, qf=nc.gpsimd)
            # hoist the ~2.7us ACT table load off the critical path
            nc.scalar.activation(wscr[:], wtile[:, 0:1], tanh)
            nc.scalar.dma_start(w2[:], w2_ext.ap())
            nc.sync.dma_start(w3[:], w3_ext.ap())
            nc.sync.dma_start(biasc[:], bias_ext.ap())
            build_dma(2, qc=nc.sync, qh=nc.sync, qf=nc.gpsimd)
            build_dma(3, qc=nc.gpsimd, qh=nc.scalar, qf=nc.gpsimd)

            # ---- PE warm-up: keep HAM busy through the input ramp ------
            ps_warm = pyp.tile([128, TILE], F32, tag="psy", name="ps_warm")

            def warm(n):
                for _ in range(n):
                    nc.tensor.matmul(
                        ps_warm[:, 0:64], wtile[:], wtile[:, 0:64],
                        start=True, stop=True,
                    )

            warm(N_WARM)

            # ---- per-chunk build ---------------------------------------
            a1_of_chunk = [None] * NCH
            a2_of_chunk = [None] * NCH
            psy_of_group = [None] * (NCH // 2)
            yt_of_group = [None] * (NCH // 2)

            def build_xt(c):
                mp = mpads[c]
                fcv = fc_tiles[c]
                xt = xts[c % 4]
                mt = mts.tile([128, 512], F32, tag="mt", name="mt")
                nc.vector.transpose(mt[:], mp[:])
                # u-sum of c across all four 32-row bands at once
                cr = mts.tile([128, 32], F32, tag="cred", name="cred")
                nc.vector.tensor_reduce(
                    cr[:],
                    mt[:, 0:256].rearrange("p (rb u) -> p rb u", u=U),
                    axis_x, op_add,
                )
                # c bands -> fc rows 0:16 (cols (b,r) b-major)
                for a in range(4):
                    nc.vector.tensor_copy(
                        fcv[0:16, 32 * a : 32 * a + 32].rearrange(
                            "p (b4 r) -> p r b4", b4=4
                        ),
                        cr[32 * a : 32 * a + 16, :].rearrange(
                            "p (r b4) -> p r b4", b4=4
                        ),
                    )
                # h bands -> xt[0:16) / xt[64:80)
                for a in range(4):
                    hb = 0 if a < 2 else 64
                    dst = xt[hb : hb + 16, :].rearrange(
                        "p (b r u) -> p b r u", b=8, r=R
                    )[:, 4 * (a & 1) : 4 * (a & 1) + 4]
                    src = mt[32 * a : 32 * a + 16, 256:512].rearrange(
                        "p (r b4 u) -> p b4 r u", b4=4, u=U
                    )
                    nc.vector.tensor_copy(dst, src)
                # fc ([c;F], 32 rows) broadcast over u -> xt[32:64)/[96:128)
                for half in range(2):
                    nc.vector.tensor_copy(
                        xt[32 + 64 * half : 64 + 64 * half, :].rearrange(
                            "p (b r u) -> p b r u", b=8, r=R
                        ),
                        fcv[:, 64 * half : 64 * half + 64]
                        .rearrange("p (b r) -> p b r", b=8)
                        .unsqueeze(3)
                        .broadcast_to((32, 8, R, U)),
                    )

            # ---- L1: 2-way row-tiled pair ------------------------------
            def pair(c, s):
                xt = xts[c % 4]
                psp = p1p.tile([128, 2, TILE], F32, tag="ps1", name="psp")
                for half in range(2):
                    pb = 64 * half
                    nc.tensor.matmul(
                        psp[:, half, :],
                        w1[pb : pb + 64, s * 128 : (s + 1) * 128],
                        xt[pb : pb + 64, :],
                        start=True, stop=True,
                    )
                return psp

            def evac_pair(c, s, psp, eng="S"):
                a1c = a1_of_chunk[c]
                if eng == "S":
                    nc.scalar.activation(
                        a1c[:, s, :, :], psp[:, :, :], relu,
                        bias=biasc[:, s : s + 1],
                    )
                else:
                    nc.vector.tensor_scalar(
                        a1c[:, s, :, :], psp[:, :, :],
                        biasc[:, s : s + 1], 0.0, op_add, op_max,
                    )

            # ---- L2: group k = (h, t), 4-MM accumulation ---------------
            # (gpsimd/Pool cannot touch PSUM on trn2, so evacuations are
            # spread over scalar/ACT and vector/DVE only)
            def l2_group(c, k, eng="S"):
                h, t = k // 4, k % 4
                a1c = a1_of_chunk[c]
                ps2 = p2p.tile([128, TILE], F32, tag="ps2", name="ps2")
                for s in range(4):
                    nc.tensor.matmul(
                        ps2[:],
                        w2[:, s, t * 128 : (t + 1) * 128],
                        a1c[:, s, h, :],
                        start=(s == 0), stop=(s == 3),
                    )
                a2c = a2_of_chunk[c]
                dst = a2c[:, h, t, :]
                bcol = biasc[:, 4 + t : 5 + t]
                if eng == "S":
                    nc.scalar.activation(dst, ps2[:], relu, bias=bcol)
                else:
                    nc.vector.tensor_scalar(
                        dst, ps2[:], bcol, 0.0, op_add, op_max
                    )

            # ---- L3: 128x32 col-tiled, 4 bands in one psum bank --------
            def l3_part(g, js):
                psy = psy_of_group[g]
                for tt in range(4):
                    for j in js:
                        a2c = a2_of_chunk[2 * g + j // 2]
                        # sim's psum group check is zero-region-coarse; the
                        # four col bands accumulate independently on HW
                        # (per-element has_written), so skip it
                        nc.tensor.matmul(
                            psy[32 * j : 32 * j + 32, :],
                            w3[:, tt, :],
                            a2c[:, j % 2, tt, :],
                            start=(tt == 0), stop=(tt == 3),
                            tile_position=(0, 32 * j),
                            skip_group_check=True,
                        )

            def tanh_half(g, half):
                psy = psy_of_group[g]
                pb = 64 * half
                yt = outs.tile([64, TILE], F32, tag=f"yt{half}", name=f"yt{half}")
                nc.scalar.activation(
                    yt[:], psy[pb : pb + 64, :], tanh,
                    bias=biasc[pb : pb + 64, 8:9],
                )
                if yt_of_group[g] is None:
                    yt_of_group[g] = [None, None]
                yt_of_group[g][half] = yt

            def emit_half(g, half, queues=None):
                queues = queues or (nc.sync, nc.sync)
                ytT = outs.tile([64, TILE], F32, tag=f"ytT{half}", name=f"ytT{half}")
                nc.vector.transpose(ytT[:], yt_of_group[g][half][:])
                for jj in range(2):
                    row0 = (4 * g + 2 * half + jj) * TILE
                    queues[jj].dma_start(
                        out_rows[row0 : row0 + TILE].rearrange(
                            "(k c) m -> c k m", c=32
                        ),
                        ytT[32 * jj : 32 * jj + 32, :].rearrange(
                            "p (k i) -> p k i", k=16
                        )[:, :, 0:M],
                    )

            # ---- steady-state chunk streams ----------------------------
            # two slots per chunk, each [pair, pair, G, G, G, G]: pairs are
            # CLUSTERED two-at-a-time because a full-128 LDWEIGHTS cannot
            # pull ahead past an in-flight row-tiled matmul -- every
            # pair<->L2 boundary pays ~95ns, so fewer boundaries win.
            # build_xt(c+1) is emitted mid-stream so the DVE FIFO never
            # parks it behind late-psum evacs; tanh halves are split across
            # adjacent chunks (A in even chunks' slot1, B at the next odd
            # chunk's slot0 head, always before that chunk's l3 reuses the
            # psy bank).
            L2_ENG = {0: "S", 1: "S", 2: "S", 3: "S",
                      4: "S", 5: "V", 6: "S", 7: "S"}
            build_xt(0)
            for c in range(NCH):
                a1_of_chunk[c] = a1p.tile(
                    [128, 4, 2, TILE], F16, tag="a1", name="a1c"
                )
                a2_of_chunk[c] = a2p.tile(
                    [128, 2, 4, TILE], F16, tag="a2", name="a2c"
                )
                # slot 0: pairs s0,s1 + G0..G3 of c-1
                if c >= 5 and c % 2 == 1:
                    g = (c - 5) // 2
                    tanh_half(g, 1)
                    emit_half(g, 1)
                psp0 = pair(c, 0)
                psp1 = pair(c, 1)
                # chunk 0's evacs all ride scalar so the DVE can run the
                # first three builds back-to-back during the ramp
                evac_pair(c, 0, psp0, eng=("S" if c == 0 else "V"))
                evac_pair(c, 1, psp1, eng="S")
                if c >= 1:
                    for k in range(4):
                        l2_group(c - 1, k, L2_ENG[k])
                if c + 1 < NCH:
                    build_xt(c + 1)
                # slot 1: pairs s2,s3 + G4..G7 of c-1 (+ l3 on odd chunks)
                if c >= 4 and c % 2 == 0:
                    g = (c - 4) // 2
                    tanh_half(g, 0)
                    emit_half(g, 0)
                psp2 = pair(c, 2)
                psp3 = pair(c, 3)
                evac_pair(c, 2, psp2, eng=("S" if c == 0 else "V"))
                evac_pair(c, 3, psp3, eng="S")
                if c >= 1:
                    for k in range(4, 8):
                        l2_group(c - 1, k, L2_ENG[k])
                if c >= 3 and c % 2 == 1:
                    g = (c - 3) // 2
                    psy_of_group[g] = pyp.tile(
                        [128, TILE], F32, tag="psy", name="psy"
                    )
                    l3_part(g, (0, 1, 2, 3))
                if c == 0:
                    warm(N_FILL)
                if c + 4 < NCH:
                    build_dma(c + 4)

            # ---- drain: l2(15), l3(7) split, tanh(6,7) -----------------
            tanh_half(6, 0)
            tanh_half(6, 1)
            emit_half(6, 0)
            emit_half(6, 1)
            psy_of_group[7] = pyp.tile([128, TILE], F32, tag="psy", name="psyF")
            l2_group(15, 0, "S")
            l2_group(15, 1, "V")
            # bands 0,1 need only chunk 14's a2 -- finish + store half A
            # while the rest of chunk 15's L2 still runs
            l3_part(7, (0, 1))
            tanh_half(7, 0)
            emit_half(7, 0)
            l2_group(15, 2, "S")
            l2_group(15, 3, "V")
            l3_part(7, (2,))
            l2_group(15, 4, "S")
            l2_group(15, 5, "V")
            l2_group(15, 6, "S")
            l2_group(15, 7, "V")
            l3_part(7, (3,))
            tanh_half(7, 1)
            emit_half(7, 1, queues=(nc.sync, nc.scalar))

    nc.compile()
    return nc


def _pack_weights(np_in):
    W1 = np.asarray(np_in["W1"], np.float32)
    W2 = np.asarray(np_in["W2"], np.float32)
    W3 = np.asarray(np_in["W3"], np.float32)
    b1 = np.asarray(np_in["b1"], np.float32)
    b2 = np.asarray(np_in["b2"], np.float32)
    b3 = np.asarray(np_in["b3"], np.float32)

    # X^T strip rows: [0:16)=h [16:32)=0 [32:48)=c [48:64)=F ; W1 rows are
    # ordered (F 0:16, c 16:32, h 32:48) in the reference
    w1p = np.zeros((128, H1), np.float16)
    w1p[0:16] = W1[32:48]
    w1p[32:48] = W1[16:32]
    w1p[48:64] = W1[0:16]
    w1p[64:128] = w1p[0:64]

    w2p = np.ascontiguousarray(
        W2.reshape(4, 128, H2).transpose(1, 0, 2).astype(np.float16)
    )
    # pad W3 to 32 out cols so each L3 col band writes its full 32 psum
    # partitions (bands 16:32 etc. would otherwise be uninitialized reads
    # for the whole-half tanh; as zeros they tanh to 0 and are dropped)
    w3p = np.zeros((128, 4, 32), np.float16)
    w3p[:, :, 0:M] = W3.reshape(4, 128, M).transpose(1, 0, 2)
    biasp = np.zeros((128, 9), np.float32)
    biasp[:, 0:4] = b1.reshape(4, 128).T
    biasp[:, 4:8] = b2.reshape(4, 128).T
    for j in range(4):
        biasp[32 * j : 32 * j + M, 8] = b3
    return {"w1p": w1p, "w2p": w2p, "w3p": w3p, "biasp": biasp}


def _core_inputs(np_in, i, packed=None):
    if packed is None:
        packed = _pack_weights(np_in)
    sl = slice(i * B_SH, (i + 1) * B_SH)
    return {
        "C": np_in["C"][sl],
        "F": np_in["F"][sl],
        "H": np_in["H"][sl],
        **packed,
    }


def _get_nc():
    if "nc" not in _CACHE:
        _CACHE["nc"] = _build()
    return _CACHE["nc"]


def run(inputs, trace=False):
    nc = _get_nc()
    np_in = {k: np.ascontiguousarray(np.asarray(v, dtype=np.float32))
             for k, v in inputs.items()}
    packed = _pack_weights(np_in)
    in_maps = [_core_inputs(np_in, i, packed) for i in range(N_CORES)]
    res = run_bass_kernel_spmd(nc, in_maps, list(range(N_CORES)), trace=trace)
    out = np.concatenate([res.results[i]["out"] for i in range(N_CORES)], axis=0)
    return out, res


def kernel(**inputs):
    out, _ = run(inputs, trace=False)
    return out
